# revision 29
# baseline (speedup 1.0000x reference)
"""Trainium2 Bass kernel: per-sample mean-pool over valid tokens + 4x head repeat.

Problem: encoded_batch [32, 2048, 1024] f32 with padding rows exactly zero,
text_lengths [32]. Output [32, 4096] = repeat(mean over valid tokens, 4).

Because padding rows are exactly zero, the masked sum equals the sum over the
first len rows, so only those rows are streamed. Samples are bin-packed onto
cores (4 per core, balancing total rows), and each core's valid rows are
host-packed into TWO contiguous streams: an fp8(e4m3) stream and an fp16
stream. The reduction is row-linear, so any subset of a sample's rows can be
quantized to fp8; a per-sample fp8 row budget keeps the worst-case output
error ~1e-2 (gate is 2e-2): n8 <= 0.0633 * L^2 / L_min, derived from
rms(e4m3 rel err)=0.0265, a 4.5-sigma tail, and max|mean| >= 3.0/sqrt(L_min).
Long samples (relative to the shortest) stream almost entirely at 1 byte/elem.

Routing is DATA-driven: each stream's matmul stationary operand is a
host-built selector sel[:, 4t+m] = 1 iff the row that block t assigns to
partition p belongs to sample slot m, so a single SPMD program accumulates
all four samples into one [4, 1024] PSUM tile. The program depends only on
(T8, T16) (cached), so it stays correct for arbitrary inputs.

The fp8 phase runs first (PE-bound at ~518ns/block); the fp16 phase last
(DMA-bound) with a tile taper chosen so the PE finishes ~1us after the last
byte. The device returns raw per-sample SUMS [4, 1024] fp16; the 1/len scale
and 4x head repeat are unshard glue done on the host.

Sharding: pure data parallel across 8 NeuronCores, no cross-core traffic.
"""

import numpy as np
import ml_dtypes

import concourse.bass as bass
import concourse.tile as tile
from concourse import bacc, mybir
from concourse.bass_utils import run_bass_kernel_spmd

B, S, D = 32, 2048, 1024
NH = 4
N_CORES = 8
BPC = B // N_CORES            # sample slots per core
P = 128

F8_NP = ml_dtypes.float8_e4m3
F8_COEF = 0.0633              # n8 budget coefficient (see module docstring)

_CACHE = {}
LAST_RESULTS = None  # BassKernelResults of the most recent kernel() call


def _split_rows(rows, quantum=128):
    """Split a packed stream into DMA tile row counts: short ramp-up so the
    PE starts while the stream is young, 1024-row tiles in the middle, and a
    fixed small-tile tail so the per-tile completion-semaphore latency never
    strands PE work after the last byte lands. All sizes are multiples of
    `quantum` (256 for the DoubleRow fp8 stream)."""
    assert rows % quantum == 0 and rows > 0
    TAIL = [512, 512, 256, 256, 128, 128] if quantum == 128 else [512, 512, 256]
    suf = []  # longest TAIL suffix that fits
    s = 0
    for sz in reversed(TAIL):
        if s + sz <= rows:
            suf.insert(0, sz)
            s += sz
        else:
            break
    body = rows - s
    out = []
    if body:
        for sz in (256, 512):  # ramp-up
            if body >= sz + 1024:
                out.append(sz)
                body -= sz
        while body >= 1024 + quantum:
            out.append(1024)
            body -= 1024
        if body:
            out.append(body)  # one filler tile <= 1024
    return out + suf


def _build(T8, T16):
    """Build the SPMD program for T8 fp8 + T16 fp16 packed 128-row blocks."""
    f32 = mybir.dt.float32
    f16 = mybir.dt.float16
    f8 = mybir.dt.float8e4
    nc = bacc.Bacc("TRN2", target_bir_lowering=False, debug=False)

    streams = []  # (x_param, sel_param, T, dtype)
    if T8:
        # DoubleRow LDWEIGHTS needs the pair's two selector column sets 16
        # elements apart (step%16==0), so sel8 uses 32 columns per block
        # pair: column 32*(t//2) + 16*(t%2) + m routes block t, slot m.
        x8 = nc.declare_dram_parameter("x8", [T8 * P, D], f8, isOutput=False)
        s8 = nc.declare_dram_parameter("sel8", [P, 16 * T8], f8, isOutput=False)
        streams.append((x8, s8, T8, f8))
    if T16:
        x16 = nc.declare_dram_parameter("x16", [T16 * P, D], f16, isOutput=False)
        s16 = nc.declare_dram_parameter("sel16", [P, NH * T16], f16, isOutput=False)
        streams.append((x16, s16, T16, f16))
    assert streams
    out = nc.declare_dram_parameter("out", [BPC, D], f16, isOutput=True)

    with tile.TileContext(nc) as tc:
        with (
            tc.tile_pool(name="xin8", bufs=8) as xpool8,
            tc.tile_pool(name="xin16", bufs=4) as xpool16,
            tc.tile_pool(name="acc", bufs=1, space="PSUM") as psum_pool,
            tc.tile_pool(name="aux", bufs=1) as aux,
        ):
            # Tiny selector loads ride the ACT HWDGE ring so they never
            # queue behind the big x-tile transfers on the sync ring.
            sels = []
            for _, sp, T, dt in streams:
                ssb = aux.tile([P, (16 if dt == f8 else NH) * T], dt)
                nc.scalar.dma_start(ssb[:], sp.ap())
                sels.append(ssb)

            # Pre-warm the ACT Copy function table so the one-time
            # LoadActFuncSet (~1.5us) overlaps the stream instead of
            # landing inside the epilogue.
            warm = aux.tile([1, 1], f16)
            nc.scalar.activation(
                warm[:], sels[-1][0:1, 0:1],
                mybir.ActivationFunctionType.Copy, scale=1.0,
            )

            # HAM warm-up: the PE clock sits at 1.2 GHz until it has been
            # continuously busy for ~3.4us. Dummy matmuls into a scratch
            # PSUM bank fill the PE's idle windows (pre-stream and the
            # delivery gaps between early tiles) so the clock reaches
            # 2.4 GHz before the bulk of the real matmuls run.
            wtile = aux.tile([P, 256], f16)
            nc.gpsimd.memset(wtile[:], 0.0)
            scratch = psum_pool.tile([BPC, 256], f32)

            def dummy_mms(n):
                for _ in range(n):
                    nc.tensor.matmul(
                        scratch[:, :], wtile[:, 0:BPC], wtile[:, 0:256],
                        start=True, stop=True,
                    )

            dummy_mms(10)
            warm_budget = 20

            ps = psum_pool.tile([BPC, D], f32)
            n_streams = len(streams)
            for si, (xp, _, T, dt) in enumerate(streams):
                sel_sb = sels[si]
                is8 = dt == f8
                tiles = _split_rows(T * P, quantum=256 if is8 else 128)
                assert sum(tiles) == T * P
                row_off = 0
                t_idx = 0
                for rows in tiles:
                    rpp = rows // P
                    src = xp.ap()[row_off : row_off + rows, :].rearrange(
                        "(p a) d -> p (a d)", p=P
                    )
                    first = si == 0 and row_off == 0
                    row_off += rows
                    last = si == n_streams - 1 and row_off == T * P
                    pool = xpool8 if is8 else xpool16
                    xt = pool.tile([P, rpp * D], dt, tag=f"xt{si}")
                    nc.sync.dma_start(xt[:], src)
                    if is8:
                        # DoubleRow: one matmul pair contracts TWO packed
                        # row-blocks (2 elem/cycle at fp8), halving PE time.
                        for r in range(0, rpp, 2):
                            g = t_idx // 2
                            w3 = sel_sb[:, 32 * g : 32 * (g + 1)].rearrange(
                                "p (two m) -> p two m", two=2
                            )[:, :, 0:NH]
                            x3 = xt[:, r * D : (r + 2) * D].rearrange(
                                "p (two d) -> p two d", two=2
                            )
                            for h in range(D // 512):
                                nc.tensor.matmul(
                                    ps[0:BPC, h * 512 : (h + 1) * 512],
                                    w3,
                                    x3[:, :, h * 512 : (h + 1) * 512],
                                    start=(first and r == 0),
                                    stop=(last and r == rpp - 2),
                                    perf_mode=mybir.MatmulPerfMode.DoubleRow,
                                )
                            t_idx += 2
                    else:
                        for r in range(rpp):
                            w = sel_sb[:, NH * t_idx : NH * (t_idx + 1)]
                            for h in range(D // 512):
                                c0 = r * D + h * 512
                                nc.tensor.matmul(
                                    ps[0:BPC, h * 512 : (h + 1) * 512],
                                    w,
                                    xt[:, c0 : c0 + 512],
                                    start=(first and r == 0),
                                    stop=(last and r == rpp - 1),
                                )
                            t_idx += 1
                    if warm_budget > 0 and not last:
                        dummy_mms(4)
                        warm_budget -= 4
                assert t_idx == T

            # Epilogue: PSUM -> SBUF fp16 copies, lower half on DVE and
            # upper half on ACT so the two PSUM banks drain in parallel;
            # each half's 4 KB output DMA rides its own HWDGE ring as soon
            # as that half's copy completes.
            h2 = D // 2
            res_sb = aux.tile([BPC, D], f16)
            nc.vector.tensor_scalar_mul(res_sb[:, 0:h2], ps[0:BPC, 0:h2], 1.0)
            nc.scalar.activation(
                res_sb[:, h2:D], ps[0:BPC, h2:D],
                mybir.ActivationFunctionType.Copy, scale=1.0,
            )
            nc.sync.dma_start(out.ap()[:, 0:h2], res_sb[:, 0:h2])
            nc.scalar.dma_start(out.ap()[:, h2:D], res_sb[:, h2:D])

    nc.compile()
    return nc


def _pack_stream(x, samples, T, np_dtype, quantum=128, dr=False):
    """Pack (sample_idx, row_start, row_count) pieces into a [T*128, D]
    stream + its per-block selector, returning (stream, sel). With dr=True
    the selector uses the DoubleRow padded layout (32 cols per block pair,
    subrow i at column offset 16*i)."""
    xp = np.zeros((T * P, D), dtype=np_dtype)
    row_slot = np.full(T * P, -1, dtype=np.int64)
    off = 0
    for m, i, r0, nr in samples:
        xp[off : off + nr] = x[i, r0 : r0 + nr]
        row_slot[off : off + nr] = m
        off += nr
    # The matmul for group index t within a [128, rpp*D] tile sums rows
    # {tile_base + p*rpp + r} (partition p owns rpp consecutive rows), so
    # route each PARTITION's actual row to its sample slot.
    selc = np.zeros((P, (16 if dr else NH) * T), dtype=np_dtype)
    pidx = np.arange(P)
    t = 0
    base = 0
    for rows_ in _split_rows(T * P, quantum=quantum):
        rpp = rows_ // P
        for r in range(rpp):
            rs = row_slot[base + pidx * rpp + r]
            valid = rs >= 0
            col0 = 32 * (t // 2) + 16 * (t % 2) if dr else NH * t
            selc[pidx[valid], col0 + rs[valid]] = 1.0
            t += 1
        base += rows_
    assert t == T
    return xp, selc


def kernel(**inputs) -> np.ndarray:
    global LAST_RESULTS
    x = np.asarray(inputs["encoded_batch"])
    lengths = np.asarray(inputs["text_lengths"]).astype(np.int64)
    assert x.shape == (B, S, D), x.shape

    nrows = np.maximum(1, lengths).astype(np.int64)
    lmin = int(nrows.min())
    # Per-sample fp8 row budget (multiples of 1; see module docstring).
    n8cap = np.minimum(nrows, (F8_COEF * nrows.astype(np.float64) ** 2 / lmin)
                       .astype(np.int64))

    # Bin-pack samples onto cores (8 bins of 4 samples) minimizing the
    # stream cost T8 + 2*T16 (bytes in 128-row blocks): greedy LPT plus
    # randomized restarts, keep best.
    def stream_cost(bins_):
        rows_c = np.array([sum(int(nrows[i]) for i in b) for b in bins_])
        cap_c = np.array([sum(int(n8cap[i]) for i in b) for b in bins_])
        best = None
        for T8 in range(0, int(cap_c.max() // P) + 3, 2):  # even: DoubleRow
            r8 = np.minimum(np.minimum(cap_c, T8 * P), rows_c)
            T16 = int(-(-(int((rows_c - r8).max())) // P))
            cost = T8 + 2 * T16
            if best is None or cost < best[0]:
                best = (cost, T8, T16)
        return best

    def pack(order):
        bins_ = [[] for _ in range(N_CORES)]
        tot_ = [0] * N_CORES
        for i in order:
            c = min(
                (c for c in range(N_CORES) if len(bins_[c]) < BPC),
                key=lambda c: (tot_[c], len(bins_[c])),
            )
            bins_[c].append(int(i))
            tot_[c] += int(nrows[i])
        return bins_

    rng = np.random.RandomState(0)
    order = np.argsort(-nrows, kind="stable")
    bins = pack(order)
    best_cost, T8, T16 = stream_cost(bins)
    for _ in range(400):
        cand = order.copy()
        a = rng.randint(0, B - 4)
        seg = cand[a : a + rng.randint(2, 12)].copy()
        rng.shuffle(seg)
        cand[a : a + len(seg)] = seg
        b2 = pack(cand)
        c2 = stream_cost(b2)
        if c2[0] < best_cost:
            best_cost, T8, T16 = c2[0], c2[1], c2[2]
            bins, order = b2, cand

    if (T8, T16) not in _CACHE:
        _CACHE[(T8, T16)] = _build(T8, T16)
    nc = _CACHE[(T8, T16)]

    in_maps = []
    for c in range(N_CORES):
        # Distribute this core's fp8 row quota over its samples (greedy,
        # longest first), respecting per-sample caps and the T8 block cap.
        idxs = sorted(bins[c], key=lambda i: -int(nrows[i]))
        quota = min(T8 * P, sum(int(n8cap[i]) for i in idxs))
        a = {}
        for i in idxs:
            take = min(int(n8cap[i]), quota)
            a[i] = take
            quota -= take
        s8_parts = []
        s16_parts = []
        for m, i in enumerate(bins[c]):
            if a[i]:
                s8_parts.append((m, i, 0, a[i]))
            rest = int(nrows[i]) - a[i]
            if rest:
                s16_parts.append((m, i, a[i], rest))
        assert sum(p[3] for p in s16_parts) <= T16 * P, (c, T8, T16)
        im = {}
        if T8:
            im["x8"], im["sel8"] = _pack_stream(x, s8_parts, T8, F8_NP,
                                                quantum=256, dr=True)
        if T16:
            im["x16"], im["sel16"] = _pack_stream(x, s16_parts, T16, np.float16)
        in_maps.append(im)
    res = run_bass_kernel_spmd(nc, in_maps, list(range(N_CORES)))
    LAST_RESULTS = res

    # Unshard glue: apply 1/len and the 4x head repeat (heads fastest-
    # varying) on the host.
    full = np.empty((B, D * NH), dtype=np.float32)
    for c in range(N_CORES):
        sums = res.results[c]["out"].astype(np.float32)  # [BPC, D] sums
        for m, i in enumerate(bins[c]):
            mean = sums[m] / np.float32(lengths[i])
            full[i] = np.repeat(mean, NH)
    return full


# revision 30
# speedup vs baseline: 1.0473x; 1.0473x over previous
"""Trainium2 Bass kernel: per-sample mean-pool over valid tokens + 4x head repeat.

Problem: encoded_batch [32, 2048, 1024] f32 with padding rows exactly zero,
text_lengths [32]. Output [32, 4096] = repeat(mean over valid tokens, 4).

Because padding rows are exactly zero, the masked sum equals the sum over the
first len rows, so only those rows are streamed. Samples are bin-packed onto
cores (4 per core, balancing total rows), and each core's valid rows are
host-packed into TWO contiguous streams: an fp8(e4m3) stream and an fp16
stream. The reduction is row-linear, so any subset of a sample's rows can be
quantized to fp8; a per-sample fp8 row budget keeps the worst-case output
error ~1e-2 (gate is 2e-2): n8 <= 0.0633 * L^2 / L_min, derived from
rms(e4m3 rel err)=0.0265, a 4.5-sigma tail, and max|mean| >= 3.0/sqrt(L_min).
Long samples (relative to the shortest) stream almost entirely at 1 byte/elem.

Routing is DATA-driven: each stream's matmul stationary operand is a
host-built selector sel[:, 4t+m] = 1 iff the row that block t assigns to
partition p belongs to sample slot m, so a single SPMD program accumulates
all four samples into one [4, 1024] PSUM tile. The program depends only on
(T8, T16) (cached), so it stays correct for arbitrary inputs.

The fp8 phase runs first (PE-bound at ~518ns/block); the fp16 phase last
(DMA-bound) with a tile taper chosen so the PE finishes ~1us after the last
byte. The device returns raw per-sample SUMS [4, 1024] fp16; the 1/len scale
and 4x head repeat are unshard glue done on the host.

Sharding: pure data parallel across 8 NeuronCores, no cross-core traffic.
"""

import numpy as np
import ml_dtypes

import concourse.bass as bass
import concourse.tile as tile
from concourse import bacc, mybir
from concourse.bass_utils import run_bass_kernel_spmd

B, S, D = 32, 2048, 1024
NH = 4
N_CORES = 8
BPC = B // N_CORES            # sample slots per core
P = 128

F8_NP = ml_dtypes.float8_e4m3
F8_COEF = 0.1                 # n8 budget coefficient (see module docstring)

_CACHE = {}
LAST_RESULTS = None  # BassKernelResults of the most recent kernel() call


def _split_rows(rows, quantum=128):
    """Split a packed stream into DMA tile row counts: short ramp-up so the
    PE starts while the stream is young, 1024-row tiles in the middle, and a
    fixed small-tile tail so the per-tile completion-semaphore latency never
    strands PE work after the last byte lands. All sizes are multiples of
    `quantum` (256 for the DoubleRow fp8 stream)."""
    assert rows % quantum == 0 and rows > 0
    TAIL = [512, 512, 256, 256, 128, 128] if quantum == 128 else [512, 512, 256]
    suf = []  # longest TAIL suffix that fits
    s = 0
    for sz in reversed(TAIL):
        if s + sz <= rows:
            suf.insert(0, sz)
            s += sz
        else:
            break
    body = rows - s
    out = []
    if body:
        for sz in (256, 512):  # ramp-up
            if body >= sz + 1024:
                out.append(sz)
                body -= sz
        while body >= 1024 + quantum:
            out.append(1024)
            body -= 1024
        if body:
            out.append(body)  # one filler tile <= 1024
    return out + suf


def _build(T8, T16):
    """Build the SPMD program for T8 fp8 + T16 fp16 packed 128-row blocks."""
    f32 = mybir.dt.float32
    f16 = mybir.dt.float16
    f8 = mybir.dt.float8e4
    nc = bacc.Bacc("TRN2", target_bir_lowering=False, debug=False)

    streams = []  # (x_param, sel_param, T, dtype)
    if T8:
        # DoubleRow LDWEIGHTS needs the pair's two selector column sets 16
        # elements apart (step%16==0), so sel8 uses 32 columns per block
        # pair: column 32*(t//2) + 16*(t%2) + m routes block t, slot m.
        x8 = nc.declare_dram_parameter("x8", [T8 * P, D], f8, isOutput=False)
        s8 = nc.declare_dram_parameter("sel8", [P, 16 * T8], f8, isOutput=False)
        streams.append((x8, s8, T8, f8))
    if T16:
        x16 = nc.declare_dram_parameter("x16", [T16 * P, D], f16, isOutput=False)
        s16 = nc.declare_dram_parameter("sel16", [P, NH * T16], f16, isOutput=False)
        streams.append((x16, s16, T16, f16))
    assert streams
    out = nc.declare_dram_parameter("out", [BPC, D], f16, isOutput=True)

    with tile.TileContext(nc) as tc:
        with (
            tc.tile_pool(name="xin8", bufs=8) as xpool8,
            tc.tile_pool(name="xin16", bufs=4) as xpool16,
            tc.tile_pool(name="acc", bufs=1, space="PSUM") as psum_pool,
            tc.tile_pool(name="aux", bufs=1) as aux,
        ):
            # Tiny selector loads ride the ACT HWDGE ring so they never
            # queue behind the big x-tile transfers on the sync ring.
            sels = []
            for _, sp, T, dt in streams:
                ssb = aux.tile([P, (16 if dt == f8 else NH) * T], dt)
                nc.scalar.dma_start(ssb[:], sp.ap())
                sels.append(ssb)

            # Pre-warm the ACT Copy function table so the one-time
            # LoadActFuncSet (~1.5us) overlaps the stream instead of
            # landing inside the epilogue.
            warm = aux.tile([1, 1], f16)
            nc.scalar.activation(
                warm[:], sels[-1][0:1, 0:1],
                mybir.ActivationFunctionType.Copy, scale=1.0,
            )

            # HAM warm-up: the PE clock sits at 1.2 GHz until it has been
            # continuously busy for ~3.4us. Dummy matmuls into a scratch
            # PSUM bank fill the PE's idle windows (pre-stream and the
            # delivery gaps between early tiles) so the clock reaches
            # 2.4 GHz before the bulk of the real matmuls run.
            wtile = aux.tile([P, 256], f16)
            nc.gpsimd.memset(wtile[:], 0.0)
            scratch = psum_pool.tile([BPC, 256], f32)

            def dummy_mms(n):
                for _ in range(n):
                    nc.tensor.matmul(
                        scratch[:, :], wtile[:, 0:BPC], wtile[:, 0:256],
                        start=True, stop=True,
                    )

            dummy_mms(10)
            warm_budget = 20

            ps = psum_pool.tile([BPC, D], f32)
            n_streams = len(streams)
            for si, (xp, _, T, dt) in enumerate(streams):
                sel_sb = sels[si]
                is8 = dt == f8
                tiles = _split_rows(T * P, quantum=256 if is8 else 128)
                assert sum(tiles) == T * P
                row_off = 0
                t_idx = 0
                for rows in tiles:
                    rpp = rows // P
                    src = xp.ap()[row_off : row_off + rows, :].rearrange(
                        "(p a) d -> p (a d)", p=P
                    )
                    first = si == 0 and row_off == 0
                    row_off += rows
                    last = si == n_streams - 1 and row_off == T * P
                    pool = xpool8 if is8 else xpool16
                    xt = pool.tile([P, rpp * D], dt, tag=f"xt{si}")
                    nc.sync.dma_start(xt[:], src)
                    if is8:
                        # DoubleRow: one matmul pair contracts TWO packed
                        # row-blocks (2 elem/cycle at fp8), halving PE time.
                        for r in range(0, rpp, 2):
                            g = t_idx // 2
                            w3 = sel_sb[:, 32 * g : 32 * (g + 1)].rearrange(
                                "p (two m) -> p two m", two=2
                            )[:, :, 0:NH]
                            x3 = xt[:, r * D : (r + 2) * D].rearrange(
                                "p (two d) -> p two d", two=2
                            )
                            for h in range(D // 512):
                                nc.tensor.matmul(
                                    ps[0:BPC, h * 512 : (h + 1) * 512],
                                    w3,
                                    x3[:, :, h * 512 : (h + 1) * 512],
                                    start=(first and r == 0),
                                    stop=(last and r == rpp - 2),
                                    perf_mode=mybir.MatmulPerfMode.DoubleRow,
                                )
                            t_idx += 2
                    else:
                        for r in range(rpp):
                            w = sel_sb[:, NH * t_idx : NH * (t_idx + 1)]
                            for h in range(D // 512):
                                c0 = r * D + h * 512
                                nc.tensor.matmul(
                                    ps[0:BPC, h * 512 : (h + 1) * 512],
                                    w,
                                    xt[:, c0 : c0 + 512],
                                    start=(first and r == 0),
                                    stop=(last and r == rpp - 1),
                                )
                            t_idx += 1
                    if warm_budget > 0 and not last:
                        dummy_mms(4)
                        warm_budget -= 4
                assert t_idx == T

            # Epilogue: PSUM -> SBUF fp16 copies, lower half on DVE and
            # upper half on ACT so the two PSUM banks drain in parallel;
            # each half's 4 KB output DMA rides its own HWDGE ring as soon
            # as that half's copy completes.
            h2 = D // 2
            res_sb = aux.tile([BPC, D], f16)
            nc.vector.tensor_scalar_mul(res_sb[:, 0:h2], ps[0:BPC, 0:h2], 1.0)
            nc.scalar.activation(
                res_sb[:, h2:D], ps[0:BPC, h2:D],
                mybir.ActivationFunctionType.Copy, scale=1.0,
            )
            nc.sync.dma_start(out.ap()[:, 0:h2], res_sb[:, 0:h2])
            nc.scalar.dma_start(out.ap()[:, h2:D], res_sb[:, h2:D])

    nc.compile()
    return nc


def _pack_stream(x, samples, T, np_dtype, quantum=128, dr=False):
    """Pack (sample_idx, row_start, row_count) pieces into a [T*128, D]
    stream + its per-block selector, returning (stream, sel). With dr=True
    the selector uses the DoubleRow padded layout (32 cols per block pair,
    subrow i at column offset 16*i)."""
    xp = np.zeros((T * P, D), dtype=np_dtype)
    row_slot = np.full(T * P, -1, dtype=np.int64)
    off = 0
    for m, i, r0, nr in samples:
        xp[off : off + nr] = x[i, r0 : r0 + nr]
        row_slot[off : off + nr] = m
        off += nr
    # The matmul for group index t within a [128, rpp*D] tile sums rows
    # {tile_base + p*rpp + r} (partition p owns rpp consecutive rows), so
    # route each PARTITION's actual row to its sample slot.
    selc = np.zeros((P, (16 if dr else NH) * T), dtype=np_dtype)
    pidx = np.arange(P)
    t = 0
    base = 0
    for rows_ in _split_rows(T * P, quantum=quantum):
        rpp = rows_ // P
        for r in range(rpp):
            rs = row_slot[base + pidx * rpp + r]
            valid = rs >= 0
            col0 = 32 * (t // 2) + 16 * (t % 2) if dr else NH * t
            selc[pidx[valid], col0 + rs[valid]] = 1.0
            t += 1
        base += rows_
    assert t == T
    return xp, selc


def kernel(**inputs) -> np.ndarray:
    global LAST_RESULTS
    x = np.asarray(inputs["encoded_batch"])
    lengths = np.asarray(inputs["text_lengths"]).astype(np.int64)
    assert x.shape == (B, S, D), x.shape

    nrows = np.maximum(1, lengths).astype(np.int64)
    lmin = int(nrows.min())
    # Per-sample fp8 row budget (multiples of 1; see module docstring).
    n8cap = np.minimum(nrows, (F8_COEF * nrows.astype(np.float64) ** 2 / lmin)
                       .astype(np.int64))

    # Bin-pack samples onto cores (8 bins of 4 samples) minimizing the
    # stream cost T8 + 2*T16 (bytes in 128-row blocks): greedy LPT plus
    # randomized restarts, keep best.
    def stream_cost(bins_):
        rows_c = np.array([sum(int(nrows[i]) for i in b) for b in bins_])
        cap_c = np.array([sum(int(n8cap[i]) for i in b) for b in bins_])
        best = None
        for T8 in range(0, int(cap_c.max() // P) + 3, 2):  # even: DoubleRow
            r8 = np.minimum(np.minimum(cap_c, T8 * P), rows_c)
            T16 = int(-(-(int((rows_c - r8).max())) // P))
            cost = T8 + 2 * T16
            if best is None or cost < best[0]:
                best = (cost, T8, T16)
        return best

    def pack(order):
        bins_ = [[] for _ in range(N_CORES)]
        tot_ = [0] * N_CORES
        for i in order:
            c = min(
                (c for c in range(N_CORES) if len(bins_[c]) < BPC),
                key=lambda c: (tot_[c], len(bins_[c])),
            )
            bins_[c].append(int(i))
            tot_[c] += int(nrows[i])
        return bins_

    rng = np.random.RandomState(0)
    order = np.argsort(-nrows, kind="stable")
    bins = pack(order)
    best_cost, T8, T16 = stream_cost(bins)
    for _ in range(400):
        cand = order.copy()
        a = rng.randint(0, B - 4)
        seg = cand[a : a + rng.randint(2, 12)].copy()
        rng.shuffle(seg)
        cand[a : a + len(seg)] = seg
        b2 = pack(cand)
        c2 = stream_cost(b2)
        if c2[0] < best_cost:
            best_cost, T8, T16 = c2[0], c2[1], c2[2]
            bins, order = b2, cand

    if (T8, T16) not in _CACHE:
        _CACHE[(T8, T16)] = _build(T8, T16)
    nc = _CACHE[(T8, T16)]

    in_maps = []
    for c in range(N_CORES):
        # Distribute this core's fp8 row quota over its samples (greedy,
        # longest first), respecting per-sample caps and the T8 block cap.
        idxs = sorted(bins[c], key=lambda i: -int(nrows[i]))
        quota = min(T8 * P, sum(int(n8cap[i]) for i in idxs))
        a = {}
        for i in idxs:
            take = min(int(n8cap[i]), quota)
            a[i] = take
            quota -= take
        s8_parts = []
        s16_parts = []
        for m, i in enumerate(bins[c]):
            if a[i]:
                s8_parts.append((m, i, 0, a[i]))
            rest = int(nrows[i]) - a[i]
            if rest:
                s16_parts.append((m, i, a[i], rest))
        assert sum(p[3] for p in s16_parts) <= T16 * P, (c, T8, T16)
        im = {}
        if T8:
            im["x8"], im["sel8"] = _pack_stream(x, s8_parts, T8, F8_NP,
                                                quantum=256, dr=True)
        if T16:
            im["x16"], im["sel16"] = _pack_stream(x, s16_parts, T16, np.float16)
        in_maps.append(im)
    res = run_bass_kernel_spmd(nc, in_maps, list(range(N_CORES)))
    LAST_RESULTS = res

    # Unshard glue: apply 1/len and the 4x head repeat (heads fastest-
    # varying) on the host.
    full = np.empty((B, D * NH), dtype=np.float32)
    for c in range(N_CORES):
        sums = res.results[c]["out"].astype(np.float32)  # [BPC, D] sums
        for m, i in enumerate(bins[c]):
            mean = sums[m] / np.float32(lengths[i])
            full[i] = np.repeat(mean, NH)
    return full
